# revision 24
# baseline (speedup 1.0000x reference)
"""MenuLoss Trainium2 kernel (v4).

Math: per batch b, cal[b] = (1/700)*sum_j amt_bj * p(x_bj) for two evals (true
ids continuous f32, pred ids rounded to integers), p a deg-446 Chebyshev series.
Factor p(x) = sum_{a<28, r<16} G[a,r] * w_a(x) * t_r(x) where the device basis
columns w_a (deg 16a) and t_r (deg r) are built by a fixed recipe: an f32
SHIFTED backbone chain sh_m = T_m + 1 via ACT Square((sqrt2)z - sqrt2) = 2(z-1)^2
(exact Chebyshev doubling, immune to f16 depth amplification), unshifted to f16
single-rounding leaf columns, plus grouped f16 DVE products for the rest.  G is
solved on host in f64 against the exact recipe polynomials (cond ~350).  amt is
folded into the t-side (f16), per-batch Grams accumulate in PSUM via fp16
TensorE matmuls (1 cyc/row vs 4 for fp32), and a signed G contraction yields
calT[b]-calP[b] directly.  Penalties ride along via ACT-accumulated tanh/relu
sums.  8-way batch data parallel, per-core scalars combined on host.
"""
import functools
import sys
import types
import numpy as np
import numpy.polynomial.chebyshev as Ch

if "antenv.axon_hooks" not in sys.modules:
    _m = types.ModuleType("antenv.axon_hooks")
    _m.get_axon_ntff_profile_hook = lambda: None
    sys.modules["antenv.axon_hooks"] = _m

import concourse.bacc as bacc
import concourse.bass as bass
import concourse.mybir as mybir
import concourse.tile as tile
from concourse.bass_utils import run_bass_kernel_spmd

AFT = mybir.ActivationFunctionType
ALU = mybir.AluOpType
F32 = mybir.dt.float32
F16 = mybir.dt.float16
I32 = mybir.dt.int32

N_CORES = 8
B, J = 512, 7 * 16 * 64          # 512 batches, 7168 elements/batch
BC = B // N_CORES                # 64 batches per core
SL = 8                           # batches per slice
NSL = BC // SL                   # 8 slices
CH = J // 128                    # 56 chunk columns per batch
C = SL * CH                      # 448 columns per slice
A, R = 28, 16                    # p = sum G[a,r] w_a t_r ; deg = 16a + r
WR = 1 + 27 + 27                 # W rows: 0=ones | 1..27 true w_a | 28..54 pred
UR = 2 * R                       # U rows: 0..15 true amt*t_r | 16..31 pred
R2C = float(np.sqrt(2.0))


# ---------------- host-side basis recipe mirror + G solve ----------------
def _xladder_ops():
    # scratch idx k = t_{k+1}; idx0 = seed t_1
    return [("dbl1", 1, 0), ("mul", 2, 1, 0), ("dbl1", 3, 1), ("mulg", 4, 3, 0, 3),
            ("dbl1", 7, 3), ("mulg", 8, 7, 0, 7)]


def _wladder_ops():
    # idx a = w_a; idx1 = psi = T_16 exact
    return [("dbl1", 2, 1), ("mul", 3, 2, 1), ("dbl1", 4, 2), ("mulg", 5, 4, 1, 3),
            ("dbl1", 8, 4), ("mulg", 9, 8, 1, 7), ("dbl1", 16, 8),
            ("mulg", 17, 16, 1, 11)]


def _mir_dbl1(z):
    p = 2.0 * Ch.chebmul(z, z)
    p[0] -= 1.0
    return p


def _run_poly(cols, ops):
    for op in ops:
        if op[0] == "dbl1":
            cols[op[1]] = _mir_dbl1(cols[op[2]])
        elif op[0] == "mul":
            cols[op[1]] = Ch.chebmul(cols[op[2]], cols[op[3]])
        else:
            _, dst, srcb, src0, w = op
            for k in range(w):
                cols[dst + k] = Ch.chebmul(cols[srcb], cols[src0 + k])
    return cols


def _solve_G(coeffs447: np.ndarray) -> np.ndarray:
    xc = {0: np.array([1.0])}
    scr = _run_poly({0: np.array([0.0, 1.0])}, _xladder_ops())
    for k in range(15):
        xc[k + 1] = scr[k]
    w = _run_poly({1: _mir_dbl1(scr[7])}, _wladder_ops())
    w[0] = np.array([1.0])
    M = np.zeros((448, 448))
    for a in range(A):
        for r in range(R):
            pr = Ch.chebmul(w[a], xc[r])
            M[: len(pr), a * R + r] = pr
    c = np.zeros(448)
    c[:447] = coeffs447
    return np.linalg.solve(M, c).reshape(A, R)


# ---------------- device kernel ----------------
def _ladder(nc, X, BB, WW, UU, seed_ap, seed_f32, wbase, ubase, bias_r2,
            bias_m1, pool_u915=False):
    """Build one side: scratch X rows {0..4} = {t1,t2,t3,t4,t8}, backbone in
    BB (f32, shifted, rolling 5 slots), W rows WW[wbase+a] (a=1..27), U rows
    UU[ubase+r] (r=1..15, r=0 = amt pre-DMA'd).  U leaves reuse folded cols:
    u_{4+k} = u4*t_k (k<=3), u_{8+k} = u_k*t8 — no t5..t7 needed at all.
    pool_u915: put the (leaf-only) u9..15 group on GPSIMD for balance."""
    k = 1.0 / 111.0
    # 5-slot rolling f32 backbone: seed->0, x2->1, x4->2, x8->3, psi->4,
    # w2->0, w4->1, w8->2, w16->3 (each overwrite is past its readers)
    slots = [0, 1, 2, 3, 4, 0, 1, 2, 3]
    nc.vector.tensor_scalar(BB[:, 0, :], seed_ap, k, 0.0, ALU.mult, ALU.add)
    nc.vector.tensor_scalar(X[:, 0, :], seed_ap, k, 1.0, ALU.mult, ALU.subtract)
    # interleave chain steps with their unshifts so rolling slots are read
    # before reuse: x2,x4,x8 -> X rows; psi -> W row 1 (ACT Copy bias -1);
    # {w2,w4} and {w8,w16} land in slots {0,1} / {2,3} for DVE pair unshifts
    unshift_after = {1: X[:, 1, :], 2: X[:, 3, :], 3: X[:, 7, :],
                     4: WW[:, wbase + 1, :]}
    for i in range(8):
        nc.scalar.activation(BB[:, slots[i + 1], :], BB[:, slots[i], :],
                             AFT.Square, scale=R2C, bias=bias_r2)
        dst = unshift_after.get(i + 1)
        if dst is not None:
            nc.scalar.activation(dst, BB[:, slots[i + 1], :], AFT.Copy,
                                 bias=-1.0)
        elif i + 1 == 6:
            nc.vector.tensor_scalar(WW[:, wbase + 2:wbase + 5:2, :],
                                    BB[:, 0:2, :], 1.0, 1.0, ALU.mult,
                                    ALU.subtract)
        elif i + 1 == 8:
            nc.vector.tensor_scalar(WW[:, wbase + 8:wbase + 17:8, :],
                                    BB[:, 2:4, :], 1.0, 1.0, ALU.mult,
                                    ALU.subtract)
    # f16 leaf products (grouped, broadcast first operand)
    def mulg(eng, dst_ap, bc_ap, in_ap, w):
        bc = bc_ap.unsqueeze(1).broadcast_to((128, w, C))
        eng.tensor_tensor(dst_ap, bc, in_ap, ALU.mult)
    v, g = nc.vector, nc.gpsimd
    ub, wb = ubase, wbase
    # scratch leaves: t3 ; t5..7 (DVE: on the u9-15 critical path)
    v.tensor_tensor(X[:, 2, :], X[:, 1, :], X[:, 0, :], ALU.mult)
    mulg(v, X[:, 4:7, :], X[:, 3, :], X[:, 0:3, :], 3)
    # U side: fold amt into backbone cols, derive the rest from folded cols
    mulg(v, UU[:, ub + 1:ub + 3, :], UU[:, ub, :], X[:, 0:2, :], 2)  # u1,u2
    v.tensor_tensor(UU[:, ub + 4, :], UU[:, ub, :], X[:, 3, :], ALU.mult)
    v.tensor_tensor(UU[:, ub + 8, :], UU[:, ub, :], X[:, 7, :], ALU.mult)
    g.tensor_tensor(UU[:, ub + 3, :], UU[:, ub + 2, :], X[:, 0, :], ALU.mult)
    mulg(g, UU[:, ub + 5:ub + 8, :], UU[:, ub + 4, :], X[:, 0:3, :], 3)
    mulg(v, UU[:, ub + 9:ub + 16, :], UU[:, ub + 8, :], X[:, 0:7, :], 7)
    # W leaves: w3 ; w5..7 ; w9..15 ; w17..23 ; w24..27 = w8*{w16..19}
    v.tensor_tensor(WW[:, wb + 3, :], WW[:, wb + 2, :], WW[:, wb + 1, :], ALU.mult)
    mulg(v, WW[:, wb + 5:wb + 8, :], WW[:, wb + 4, :], WW[:, wb + 1:wb + 4, :], 3)
    mulg(v, WW[:, wb + 9:wb + 16, :], WW[:, wb + 8, :], WW[:, wb + 1:wb + 8, :], 7)
    mulg(g if pool_u915 else v, WW[:, wb + 17:wb + 28, :], WW[:, wb + 16, :], WW[:, wb + 1:wb + 12, :], 11)


def _build(slices=NSL):
    nc = bacc.Bacc("TRN2", target_bir_lowering=False, debug=False, num_devices=1)
    ip = nc.dram_tensor("ip", [BC, J], F32, kind="ExternalInput")   # pred ids raw
    pa = nc.dram_tensor("pa", [BC, J], F16, kind="ExternalInput")   # pred amt
    it = nc.dram_tensor("it", [BC, J], F32, kind="ExternalInput")   # true ids
    ta = nc.dram_tensor("ta", [BC, J], F16, kind="ExternalInput")   # true amt
    gc = nc.dram_tensor("gc", [WR, SL * UR], F32, kind="ExternalInput")
    out = nc.dram_tensor("out", [1, 8], F32, kind="ExternalOutput")

    bias_np = np.broadcast_to(
        np.array([-np.sqrt(2.0), -222.0, -1.0], np.float32), (128, 3)).copy()
    bias_dram = nc.inline_tensor(bias_np, name="bias_const")

    ip_r = ip.ap().rearrange("b (p c) -> p b c", p=128)
    pa_r = pa.ap().rearrange("b (p c) -> p b c", p=128)
    it_r = it.ap().rearrange("b (p c) -> p b c", p=128)
    ta_r = ta.ap().rearrange("b (p c) -> p b c", p=128)

    with tile.TileContext(nc) as tc:
        with (
            tc.tile_pool(name="data", bufs=2) as data_pool,
            tc.tile_pool(name="basis", bufs=2) as basis_pool,
            tc.tile_pool(name="scr", bufs=1) as scr_pool,
            tc.tile_pool(name="small", bufs=1) as small_pool,
            tc.tile_pool(name="psum", bufs=2, space="PSUM") as psum_pool,
            tc.tile_pool(name="psc", bufs=2, space="PSUM") as psc_pool,
        ):
            gct = small_pool.tile([WR, SL * UR], F32)
            nc.sync.dma_start(gct[:], gc.ap())
            bias_t = small_pool.tile([128, 3], F32)
            nc.sync.dma_start(bias_t[:], bias_dram.ap())
            ones55 = small_pool.tile([WR, 1], F16)
            nc.gpsimd.memset(ones55[:], 1.0)
            ones128 = small_pool.tile([128, 1], F16)
            nc.gpsimd.memset(ones128[:], 1.0)
            diffs = small_pool.tile([1, BC], F32)
            pen_i = small_pool.tile([128, NSL], F32)
            pen_a = small_pool.tile([128, NSL], F32)
            pen_m = small_pool.tile([128, NSL], F32)
            pen_r = small_pool.tile([128, NSL], F32)

            for s in range(slices):
                bs = slice(s * SL, (s + 1) * SL)
                PI = data_pool.tile([128, C], F32, tag="PI")
                TI = data_pool.tile([128, C], F32, tag="TI")
                WW = basis_pool.tile([128, WR, C], F16, tag="WW")
                UU = basis_pool.tile([128, UR, C], F16, tag="UU")
                r3 = lambda ap_: ap_.rearrange("p (b c) -> p b c", b=SL)
                nc.sync.dma_start(r3(PI[:]), ip_r[:, bs, :])
                nc.sync.dma_start(r3(TI[:]), it_r[:, bs, :])
                nc.sync.dma_start(r3(UU[:, 0, :]), ta_r[:, bs, :])
                nc.sync.dma_start(r3(UU[:, R, :]), pa_r[:, bs, :])

                XT = scr_pool.tile([128, 8, C], F16, tag="XT")
                XP = scr_pool.tile([128, 8, C], F16, tag="XP")
                BT = scr_pool.tile([128, 5, C], F32, tag="BT")
                BP = scr_pool.tile([128, 5, C], F32, tag="BP")
                nc.gpsimd.memset(WW[:, 0, :], 1.0)

                # penalties on raw pred (tanh/relu accumulate on ACT)
                ti = scr_pool.tile([128, C], F16, tag="ti")
                ta_t = scr_pool.tile([128, C], F16, tag="ta")
                tm = scr_pool.tile([128, C], F16, tag="tm")
                nc.scalar.activation(ti[:], PI[:], AFT.Tanh, scale=4.0,
                                     accum_out=pen_i[:, s:s + 1])
                nc.scalar.activation(ta_t[:], UU[:, R, :], AFT.Tanh, scale=4.0,
                                     accum_out=pen_a[:, s:s + 1])
                nc.vector.scalar_tensor_tensor(tm[:], ti[:], 1.0, ta_t[:],
                                               ALU.mult, ALU.mult,
                                               accum_out=pen_m[:, s:s + 1])
                rl = scr_pool.tile([128, C], F16, tag="rl")
                nc.scalar.activation(rl[:], PI[:], AFT.Relu, bias=bias_t[:, 1:2],
                                     accum_out=pen_r[:, s:s + 1])

                # round pred ids (rte int convert, on Pool)
                ki = scr_pool.tile([128, C], I32, tag="ki")
                kf = scr_pool.tile([128, C], F16, tag="kf")
                nc.vector.tensor_copy(ki[:], PI[:])
                nc.vector.tensor_copy(kf[:], ki[:])

                _ladder(nc, XT, BT, WW, UU, TI[:], True, 0, 0, bias_t[:, 0:1],
                        bias_t[:, 2:3], pool_u915=False)
                _ladder(nc, XP, BP, WW, UU, kf[:], False, 27, R,
                        bias_t[:, 0:1], bias_t[:, 2:3], pool_u915=True)

                # per-batch Grams: accumulate CH chunks into PSUM
                ps = psum_pool.tile([WR, SL * UR], F32, tag="gram")
                for b in range(SL):
                    for cc in range(CH):
                        j = b * CH + cc
                        nc.tensor.matmul(ps[:, b * UR:(b + 1) * UR],
                                         WW[:, :, j], UU[:, :, j],
                                         start=(cc == 0), stop=(cc == CH - 1))
                gs = scr_pool.tile([WR, SL * UR], F16, tag="gs")
                nc.vector.scalar_tensor_tensor(gs[:], ps[:], 1.0, gct[:],
                                               ALU.mult, ALU.mult)
                ps2 = psc_pool.tile([1, SL * UR], F32, tag="colsum")
                nc.tensor.matmul(ps2[:], ones55[:], gs[:], start=True, stop=True)
                sall = scr_pool.tile([1, SL * UR], F32, tag="sall")
                nc.scalar.copy(sall[:], ps2[:])
                nc.vector.tensor_reduce(
                    diffs[:, s * SL:(s + 1) * SL],
                    sall[:].rearrange("p (b n) -> p b n", n=UR),
                    mybir.AxisListType.X, ALU.add)

            # final: v0 = sum_b diffs^2 ; penalty partition sums
            dsq = small_pool.tile([1, BC], F32)
            nc.scalar.activation(dsq[:], diffs[:], AFT.Square)
            v0 = small_pool.tile([1, 1], F32)
            nc.vector.tensor_reduce(v0[:], dsq[:], mybir.AxisListType.X, ALU.add)
            pen_red = small_pool.tile([128, 4], F16)
            with nc.allow_low_precision(reason="penalty sums are O(10) scalars"):
                for idx, t in enumerate((pen_i, pen_a, pen_m, pen_r)):
                    nc.vector.tensor_reduce(pen_red[:, idx:idx + 1], t[:],
                                            mybir.AxisListType.X, ALU.add)
            ps3 = psc_pool.tile([1, 4], F32, tag="pen")
            nc.tensor.matmul(ps3[:], ones128[:], pen_red[:], start=True, stop=True)
            ot = small_pool.tile([1, 8], F32)
            nc.vector.tensor_copy(ot[:, 0:1], v0[:])
            nc.vector.tensor_copy(ot[:, 1:5], ps3[:])
            nc.gpsimd.memset(ot[:, 5:8], 0.0)
            nc.sync.dma_start(out.ap(), ot[:])
    nc.compile()
    return nc


@functools.lru_cache(maxsize=2)
def _compiled():
    return _build()


def kernel(y_pred: np.ndarray, y: np.ndarray, calories_coeffs: np.ndarray,
           _trace: bool = False):
    G = _solve_G(np.asarray(calories_coeffs, np.float64)) / 700.0
    gcv = np.zeros((WR, SL * UR), np.float32)
    for b in range(SL):
        blk = gcv[:, b * UR:(b + 1) * UR]
        blk[0, 0:R] = G[0]
        blk[1:A, 0:R] = G[1:]
        blk[0, R:2 * R] = -G[0]
        blk[A:WR, R:2 * R] = -G[1:]

    yp = np.asarray(y_pred, np.float32).reshape(B, J, 2)
    yt = np.asarray(y, np.float32).reshape(B, J, 2)
    ip_h = np.ascontiguousarray(yp[:, :, 0])
    pa_h = np.ascontiguousarray(yp[:, :, 1].astype(np.float16))
    it_h = np.ascontiguousarray(yt[:, :, 0])
    ta_h = np.ascontiguousarray(yt[:, :, 1].astype(np.float16))
    in_maps = []
    for i in range(N_CORES):
        sl_ = slice(i * BC, (i + 1) * BC)
        in_maps.append({"ip": ip_h[sl_], "pa": pa_h[sl_], "it": it_h[sl_],
                        "ta": ta_h[sl_], "gc": gcv})
    nc = _compiled()
    res = run_bass_kernel_spmd(nc, in_maps, list(range(N_CORES)), trace=_trace)
    parts = np.stack([r["out"][0] for r in res.results])  # [8, 8]
    tot = parts.sum(axis=0).astype(np.float64)
    v0, a1, a2, a3, rl = tot[0], tot[1], tot[2], tot[3], tot[4]
    loss = (v0 + (a1 + a2 - 2.0 * a3) + rl) / float(B)
    outv = np.float32(loss)
    if _trace:
        return outv, res
    return outv


# revision 25
# speedup vs baseline: 1.2663x; 1.2663x over previous
"""MenuLoss Trainium2 kernel (v4).

Math: per batch b, cal[b] = (1/700)*sum_j amt_bj * p(x_bj) for two evals (true
ids continuous f32, pred ids rounded to integers), p a deg-446 Chebyshev series.
Factor p(x) = sum_{a<28, r<16} G[a,r] * w_a(x) * t_r(x) where the device basis
columns w_a (deg 16a) and t_r (deg r) are built by a fixed recipe: an f32
SHIFTED backbone chain sh_m = T_m + 1 via ACT Square((sqrt2)z - sqrt2) = 2(z-1)^2
(exact Chebyshev doubling, immune to f16 depth amplification), unshifted to f16
single-rounding leaf columns, plus grouped f16 DVE products for the rest.  G is
solved on host in f64 against the exact recipe polynomials (cond ~350).  amt is
folded into the t-side (f16), per-batch Grams accumulate in PSUM via fp16
TensorE matmuls (1 cyc/row vs 4 for fp32), and a signed G contraction yields
calT[b]-calP[b] directly.  Penalties ride along via ACT-accumulated tanh/relu
sums.  8-way batch data parallel, per-core scalars combined on host.
"""
import functools
import sys
import types
import numpy as np
import numpy.polynomial.chebyshev as Ch

if "antenv.axon_hooks" not in sys.modules:
    _m = types.ModuleType("antenv.axon_hooks")
    _m.get_axon_ntff_profile_hook = lambda: None
    sys.modules["antenv.axon_hooks"] = _m

import concourse.bacc as bacc
import concourse.bass as bass
import concourse.mybir as mybir
import concourse.tile as tile
from concourse.bass_utils import run_bass_kernel_spmd

AFT = mybir.ActivationFunctionType
ALU = mybir.AluOpType
F32 = mybir.dt.float32
F16 = mybir.dt.float16
I32 = mybir.dt.int32

N_CORES = 8
B, J = 512, 7 * 16 * 64          # 512 batches, 7168 elements/batch
BC = B // N_CORES                # 64 batches per core
SL = 8                           # batches per slice
NSL = BC // SL                   # 8 slices
CH = J // 128                    # 56 chunk columns per batch
C = SL * CH                      # 448 columns per slice
A, R = 28, 16                    # p = sum G[a,r] w_a t_r ; deg = 16a + r
WR = 1 + 27 + 27                 # W rows: 0=ones | 1..27 true w_a | 28..54 pred
UR = 2 * R                       # U rows: 0..15 true amt*t_r | 16..31 pred
R2C = float(np.sqrt(2.0))


# ---------------- host-side basis recipe mirror + G solve ----------------
def _xladder_ops():
    # scratch idx k = t_{k+1}; idx0 = seed t_1
    return [("dbl1", 1, 0), ("mul", 2, 1, 0), ("dbl1", 3, 1), ("mulg", 4, 3, 0, 3),
            ("dbl1", 7, 3), ("mulg", 8, 7, 0, 7)]


def _wladder_ops():
    # idx a = w_a; idx1 = psi = T_16 exact
    return [("dbl1", 2, 1), ("mul", 3, 2, 1), ("dbl1", 4, 2), ("mulg", 5, 4, 1, 3),
            ("dbl1", 8, 4), ("mulg", 9, 8, 1, 7), ("dbl1", 16, 8),
            ("mulg", 17, 16, 1, 11)]


def _mir_dbl1(z):
    p = 2.0 * Ch.chebmul(z, z)
    p[0] -= 1.0
    return p


def _run_poly(cols, ops):
    for op in ops:
        if op[0] == "dbl1":
            cols[op[1]] = _mir_dbl1(cols[op[2]])
        elif op[0] == "mul":
            cols[op[1]] = Ch.chebmul(cols[op[2]], cols[op[3]])
        else:
            _, dst, srcb, src0, w = op
            for k in range(w):
                cols[dst + k] = Ch.chebmul(cols[srcb], cols[src0 + k])
    return cols


def _solve_G(coeffs447: np.ndarray) -> np.ndarray:
    xc = {0: np.array([1.0])}
    scr = _run_poly({0: np.array([0.0, 1.0])}, _xladder_ops())
    for k in range(15):
        xc[k + 1] = scr[k]
    w = _run_poly({1: _mir_dbl1(scr[7])}, _wladder_ops())
    w[0] = np.array([1.0])
    M = np.zeros((448, 448))
    for a in range(A):
        for r in range(R):
            pr = Ch.chebmul(w[a], xc[r])
            M[: len(pr), a * R + r] = pr
    c = np.zeros(448)
    c[:447] = coeffs447
    return np.linalg.solve(M, c).reshape(A, R)


# ---------------- device kernel ----------------
def _ladder(nc, X, BB, WW, UU, seed_ap, seed_f32, wbase, ubase, bias_r2,
            bias_m1, pool_u915=False):
    """Build one side: scratch X rows {0..4} = {t1,t2,t3,t4,t8}, backbone in
    BB (f32, shifted, rolling 5 slots), W rows WW[wbase+a] (a=1..27), U rows
    UU[ubase+r] (r=1..15, r=0 = amt pre-DMA'd).  U leaves reuse folded cols:
    u_{4+k} = u4*t_k (k<=3), u_{8+k} = u_k*t8 — no t5..t7 needed at all.
    pool_u915: put the (leaf-only) u9..15 group on GPSIMD for balance."""
    k = 1.0 / 111.0
    # 5-slot rolling f32 backbone: seed->0, x2->1, x4->2, x8->3, psi->4,
    # w2->0, w4->1, w8->2, w16->3 (each overwrite is past its readers)
    slots = [0, 1, 2, 3, 4, 0, 1, 2, 3]
    nc.vector.tensor_scalar(BB[:, 0, :], seed_ap, k, 0.0, ALU.mult, ALU.add)
    nc.vector.tensor_scalar(X[:, 0, :], seed_ap, k, 1.0, ALU.mult, ALU.subtract)
    # interleave chain steps with their unshifts so rolling slots are read
    # before reuse: x2,x4,x8 -> X rows; psi -> W row 1 (ACT Copy bias -1);
    # {w2,w4} and {w8,w16} land in slots {0,1} / {2,3} for DVE pair unshifts
    unshift_after = {1: X[:, 1, :], 2: X[:, 3, :], 3: X[:, 7, :],
                     4: WW[:, wbase + 1, :]}
    for i in range(8):
        nc.scalar.activation(BB[:, slots[i + 1], :], BB[:, slots[i], :],
                             AFT.Square, scale=R2C, bias=bias_r2)
        dst = unshift_after.get(i + 1)
        if dst is not None:
            nc.scalar.activation(dst, BB[:, slots[i + 1], :], AFT.Copy,
                                 bias=-1.0)
        elif i + 1 == 6:
            nc.scalar.activation(WW[:, wbase + 2, :], BB[:, 0, :], AFT.Copy,
                                 bias=-1.0)
            nc.scalar.activation(WW[:, wbase + 4, :], BB[:, 1, :], AFT.Copy,
                                 bias=-1.0)
        elif i + 1 == 8:
            nc.scalar.activation(WW[:, wbase + 8, :], BB[:, 2, :], AFT.Copy,
                                 bias=-1.0)
            nc.scalar.activation(WW[:, wbase + 16, :], BB[:, 3, :], AFT.Copy,
                                 bias=-1.0)
    # f16 leaf products (grouped, broadcast first operand)
    def mulg(eng, dst_ap, bc_ap, in_ap, w):
        bc = bc_ap.unsqueeze(1).broadcast_to((128, w, C))
        eng.tensor_tensor(dst_ap, bc, in_ap, ALU.mult)
    v, g = nc.vector, nc.gpsimd
    ub, wb = ubase, wbase
    # scratch leaves: t3 ; t5..7 (DVE: on the u9-15 critical path)
    v.tensor_tensor(X[:, 2, :], X[:, 1, :], X[:, 0, :], ALU.mult)
    mulg(v, X[:, 4:7, :], X[:, 3, :], X[:, 0:3, :], 3)
    # U side: fold amt into backbone cols, derive the rest from folded cols
    mulg(v, UU[:, ub + 1:ub + 3, :], UU[:, ub, :], X[:, 0:2, :], 2)  # u1,u2
    v.tensor_tensor(UU[:, ub + 4, :], UU[:, ub, :], X[:, 3, :], ALU.mult)
    v.tensor_tensor(UU[:, ub + 8, :], UU[:, ub, :], X[:, 7, :], ALU.mult)
    g.tensor_tensor(UU[:, ub + 3, :], UU[:, ub + 2, :], X[:, 0, :], ALU.mult)
    mulg(g, UU[:, ub + 5:ub + 8, :], UU[:, ub + 4, :], X[:, 0:3, :], 3)
    mulg(v, UU[:, ub + 9:ub + 16, :], UU[:, ub + 8, :], X[:, 0:7, :], 7)
    # W leaves: w3 ; w5..7 ; w9..15 ; w17..23 ; w24..27 = w8*{w16..19}
    v.tensor_tensor(WW[:, wb + 3, :], WW[:, wb + 2, :], WW[:, wb + 1, :], ALU.mult)
    mulg(v, WW[:, wb + 5:wb + 8, :], WW[:, wb + 4, :], WW[:, wb + 1:wb + 4, :], 3)
    mulg(v, WW[:, wb + 9:wb + 16, :], WW[:, wb + 8, :], WW[:, wb + 1:wb + 8, :], 7)
    mulg(v, WW[:, wb + 17:wb + 28, :], WW[:, wb + 16, :], WW[:, wb + 1:wb + 12, :], 11)


def _build(slices=NSL):
    nc = bacc.Bacc("TRN2", target_bir_lowering=False, debug=False, num_devices=1)
    ip = nc.dram_tensor("ip", [BC, J], F32, kind="ExternalInput")   # pred ids raw
    pa = nc.dram_tensor("pa", [BC, J], F16, kind="ExternalInput")   # pred amt
    it = nc.dram_tensor("it", [BC, J], F32, kind="ExternalInput")   # true ids
    ta = nc.dram_tensor("ta", [BC, J], F16, kind="ExternalInput")   # true amt
    gc = nc.dram_tensor("gc", [WR, SL * UR], F32, kind="ExternalInput")
    out = nc.dram_tensor("out", [1, 8], F32, kind="ExternalOutput")

    bias_np = np.broadcast_to(
        np.array([-np.sqrt(2.0), -222.0, -1.0], np.float32), (128, 3)).copy()
    bias_dram = nc.inline_tensor(bias_np, name="bias_const")

    ip_r = ip.ap().rearrange("b (p c) -> p b c", p=128)
    pa_r = pa.ap().rearrange("b (p c) -> p b c", p=128)
    it_r = it.ap().rearrange("b (p c) -> p b c", p=128)
    ta_r = ta.ap().rearrange("b (p c) -> p b c", p=128)

    with tile.TileContext(nc) as tc:
        with (
            tc.tile_pool(name="data", bufs=2) as data_pool,
            tc.tile_pool(name="basis", bufs=2) as basis_pool,
            tc.tile_pool(name="scr", bufs=1) as scr_pool,
            tc.tile_pool(name="small", bufs=1) as small_pool,
            tc.tile_pool(name="psum", bufs=2, space="PSUM") as psum_pool,
            tc.tile_pool(name="psc", bufs=2, space="PSUM") as psc_pool,
        ):
            gct = small_pool.tile([WR, SL * UR], F32)
            nc.sync.dma_start(gct[:], gc.ap())
            bias_t = small_pool.tile([128, 3], F32)
            nc.sync.dma_start(bias_t[:], bias_dram.ap())
            ones55 = small_pool.tile([WR, 1], F16)
            nc.gpsimd.memset(ones55[:], 1.0)
            ones128 = small_pool.tile([128, 1], F16)
            nc.gpsimd.memset(ones128[:], 1.0)
            diffs = small_pool.tile([1, BC], F32)
            pen_i = small_pool.tile([128, NSL], F32)
            pen_a = small_pool.tile([128, NSL], F32)
            pen_m = small_pool.tile([128, NSL], F32)
            pen_r = small_pool.tile([128, NSL], F32)

            for s in range(slices):
                bs = slice(s * SL, (s + 1) * SL)
                PI = data_pool.tile([128, C], F32, tag="PI")
                TI = data_pool.tile([128, C], F32, tag="TI")
                WW = basis_pool.tile([128, WR, C], F16, tag="WW")
                UU = basis_pool.tile([128, UR, C], F16, tag="UU")
                r3 = lambda ap_: ap_.rearrange("p (b c) -> p b c", b=SL)
                nc.sync.dma_start(r3(PI[:]), ip_r[:, bs, :])
                nc.sync.dma_start(r3(TI[:]), it_r[:, bs, :])
                nc.sync.dma_start(r3(UU[:, 0, :]), ta_r[:, bs, :])
                nc.sync.dma_start(r3(UU[:, R, :]), pa_r[:, bs, :])

                XT = scr_pool.tile([128, 8, C], F16, tag="XT")
                XP = scr_pool.tile([128, 8, C], F16, tag="XP")
                BT = scr_pool.tile([128, 5, C], F32, tag="BT")
                BP = scr_pool.tile([128, 5, C], F32, tag="BP")
                nc.gpsimd.memset(WW[:, 0, :], 1.0)

                # penalties on raw pred (tanh/relu accumulate on ACT)
                ti = scr_pool.tile([128, C], F16, tag="ti")
                ta_t = scr_pool.tile([128, C], F16, tag="ta")
                tm = scr_pool.tile([128, C], F16, tag="tm")
                nc.scalar.activation(ti[:], PI[:], AFT.Tanh, scale=4.0,
                                     accum_out=pen_i[:, s:s + 1])
                nc.scalar.activation(ta_t[:], UU[:, R, :], AFT.Tanh, scale=4.0,
                                     accum_out=pen_a[:, s:s + 1])
                nc.vector.scalar_tensor_tensor(tm[:], ti[:], 1.0, ta_t[:],
                                               ALU.mult, ALU.mult,
                                               accum_out=pen_m[:, s:s + 1])
                rl = scr_pool.tile([128, C], F16, tag="rl")
                nc.scalar.activation(rl[:], PI[:], AFT.Relu, bias=bias_t[:, 1:2],
                                     accum_out=pen_r[:, s:s + 1])

                # round pred ids (rte int convert, on Pool)
                ki = scr_pool.tile([128, C], I32, tag="ki")
                kf = scr_pool.tile([128, C], F16, tag="kf")
                nc.vector.tensor_copy(ki[:], PI[:])
                nc.vector.tensor_copy(kf[:], ki[:])

                _ladder(nc, XT, BT, WW, UU, TI[:], True, 0, 0, bias_t[:, 0:1],
                        bias_t[:, 2:3], pool_u915=False)
                _ladder(nc, XP, BP, WW, UU, kf[:], False, 27, R,
                        bias_t[:, 0:1], bias_t[:, 2:3], pool_u915=True)

                # per-batch Grams: accumulate CH chunks into PSUM
                ps = psum_pool.tile([WR, SL * UR], F32, tag="gram")
                for b in range(SL):
                    for cc in range(CH):
                        j = b * CH + cc
                        nc.tensor.matmul(ps[:, b * UR:(b + 1) * UR],
                                         WW[:, :, j], UU[:, :, j],
                                         start=(cc == 0), stop=(cc == CH - 1))
                gs = scr_pool.tile([WR, SL * UR], F16, tag="gs")
                nc.vector.scalar_tensor_tensor(gs[:], ps[:], 1.0, gct[:],
                                               ALU.mult, ALU.mult)
                ps2 = psc_pool.tile([1, SL * UR], F32, tag="colsum")
                nc.tensor.matmul(ps2[:], ones55[:], gs[:], start=True, stop=True)
                sall = scr_pool.tile([1, SL * UR], F32, tag="sall")
                nc.scalar.copy(sall[:], ps2[:])
                nc.vector.tensor_reduce(
                    diffs[:, s * SL:(s + 1) * SL],
                    sall[:].rearrange("p (b n) -> p b n", n=UR),
                    mybir.AxisListType.X, ALU.add)

            # final: v0 = sum_b diffs^2 ; penalty partition sums
            dsq = small_pool.tile([1, BC], F32)
            nc.scalar.activation(dsq[:], diffs[:], AFT.Square)
            v0 = small_pool.tile([1, 1], F32)
            nc.vector.tensor_reduce(v0[:], dsq[:], mybir.AxisListType.X, ALU.add)
            pen_red = small_pool.tile([128, 4], F16)
            with nc.allow_low_precision(reason="penalty sums are O(10) scalars"):
                for idx, t in enumerate((pen_i, pen_a, pen_m, pen_r)):
                    nc.vector.tensor_reduce(pen_red[:, idx:idx + 1], t[:],
                                            mybir.AxisListType.X, ALU.add)
            ps3 = psc_pool.tile([1, 4], F32, tag="pen")
            nc.tensor.matmul(ps3[:], ones128[:], pen_red[:], start=True, stop=True)
            ot = small_pool.tile([1, 8], F32)
            nc.vector.tensor_copy(ot[:, 0:1], v0[:])
            nc.vector.tensor_copy(ot[:, 1:5], ps3[:])
            nc.gpsimd.memset(ot[:, 5:8], 0.0)
            nc.sync.dma_start(out.ap(), ot[:])
    nc.compile()
    return nc


@functools.lru_cache(maxsize=2)
def _compiled():
    return _build()


def kernel(y_pred: np.ndarray, y: np.ndarray, calories_coeffs: np.ndarray,
           _trace: bool = False):
    G = _solve_G(np.asarray(calories_coeffs, np.float64)) / 700.0
    gcv = np.zeros((WR, SL * UR), np.float32)
    for b in range(SL):
        blk = gcv[:, b * UR:(b + 1) * UR]
        blk[0, 0:R] = G[0]
        blk[1:A, 0:R] = G[1:]
        blk[0, R:2 * R] = -G[0]
        blk[A:WR, R:2 * R] = -G[1:]

    yp = np.asarray(y_pred, np.float32).reshape(B, J, 2)
    yt = np.asarray(y, np.float32).reshape(B, J, 2)
    ip_h = np.ascontiguousarray(yp[:, :, 0])
    pa_h = np.ascontiguousarray(yp[:, :, 1].astype(np.float16))
    it_h = np.ascontiguousarray(yt[:, :, 0])
    ta_h = np.ascontiguousarray(yt[:, :, 1].astype(np.float16))
    in_maps = []
    for i in range(N_CORES):
        sl_ = slice(i * BC, (i + 1) * BC)
        in_maps.append({"ip": ip_h[sl_], "pa": pa_h[sl_], "it": it_h[sl_],
                        "ta": ta_h[sl_], "gc": gcv})
    nc = _compiled()
    res = run_bass_kernel_spmd(nc, in_maps, list(range(N_CORES)), trace=_trace)
    parts = np.stack([r["out"][0] for r in res.results])  # [8, 8]
    tot = parts.sum(axis=0).astype(np.float64)
    v0, a1, a2, a3, rl = tot[0], tot[1], tot[2], tot[3], tot[4]
    loss = (v0 + (a1 + a2 - 2.0 * a3) + rl) / float(B)
    outv = np.float32(loss)
    if _trace:
        return outv, res
    return outv
